# revision 10
# baseline (speedup 1.0000x reference)
"""Trainium2 Bass kernel for nn_CCorrM (co-correlation attention + dsconv).

Full-input contract: kernel(**inputs) takes the unsharded numpy inputs and
returns (exemplar_out, query_out), each [16, 128, 128, 128] float32.

Strategy: pure data parallel over batch B=16 across 8 NeuronCores
(2 samples per core); all params replicated.

Per-sample on-core pipeline (C=128, N=H*W=16384):
  G^T      = sum_n ex[:,n] q[:,n]^T           (PE transposes + fp32 matmuls)
  A        = G W_e^T, A^T = W_e G^T           (two small fp32 matmuls)
  softmax  rows of A and A^T (exp via ACT with accum row-sum; normalization
           folded into the attention-output eviction)
  y0       = softmax-att @ {ex|q} + residual  (fp32r matmuls, STT eviction)
  dwconv   9 diagonal fp32r matmuls accumulating in PSUM (3x3 depthwise,
           BN folded into tap weights host-side)
  prelu    = max(alpha*x, x) via scalar_tensor_tensor
  pwconv   1x1 conv = fp32r matmul (BN folded host-side), bias via ACT, prelu
"""

import sys

if '/opt/trn_rl_repo' not in sys.path:
    sys.path.insert(0, '/opt/trn_rl_repo')

import numpy as np

B, C, H, W = 16, 128, 128, 128
N = H * W
NCORES = 8
SPC = B // NCORES          # samples per core
STRIP = 8                  # output rows per strip
NSTRIP = H // STRIP
EPS = 1e-5

_cache = {}


def _build():
    import concourse.bacc as bacc
    import concourse.tile as tile
    from concourse import mybir
    from concourse.alu_op_type import AluOpType

    f32 = mybir.dt.float32
    f32r = mybir.dt.float32r
    Exp = mybir.ActivationFunctionType.Exp
    Ident = mybir.ActivationFunctionType.Identity
    X = mybir.AxisListType.X

    nc = bacc.Bacc("TRN2", target_bir_lowering=False, debug=False,
                   num_devices=NCORES)

    ex_d = nc.dram_tensor("ex", [SPC, C, N], f32r, kind="ExternalInput")
    q_d = nc.dram_tensor("q", [SPC, C, N], f32r, kind="ExternalInput")
    weT_d = nc.dram_tensor("weT", [C, C], f32, kind="ExternalInput")
    ident_d = nc.dram_tensor("ident", [C, C], f32, kind="ExternalInput")
    dwdiag_d = nc.dram_tensor("dwdiag", [9, C, C], f32, kind="ExternalInput")
    pwT_d = nc.dram_tensor("pwT", [C, C], f32, kind="ExternalInput")
    dwb_d = nc.dram_tensor("dwb", [C, 1], f32, kind="ExternalInput")
    dwa_d = nc.dram_tensor("dwa", [C, 1], f32, kind="ExternalInput")
    pwb_d = nc.dram_tensor("pwb", [C, 1], f32, kind="ExternalInput")
    pwa_d = nc.dram_tensor("pwa", [C, 1], f32, kind="ExternalInput")
    oex_d = nc.dram_tensor("oex", [SPC, C, N], f32, kind="ExternalOutput")
    oq_d = nc.dram_tensor("oq", [SPC, C, N], f32, kind="ExternalOutput")

    with tile.TileContext(nc) as tc:
        from contextlib import ExitStack
        with ExitStack() as ctx:
            const = ctx.enter_context(tc.tile_pool(name="const", bufs=1))
            big = ctx.enter_context(tc.tile_pool(name="big", bufs=1))
            xtsb = ctx.enter_context(tc.tile_pool(name="xtsb", bufs=3))
            smp = ctx.enter_context(tc.tile_pool(name="smp", bufs=2))
            y0p = ctx.enter_context(tc.tile_pool(name="y0p", bufs=2))
            convp = ctx.enter_context(tc.tile_pool(name="convp", bufs=2))
            outp = ctx.enter_context(tc.tile_pool(name="outp", bufs=2))

            weT = const.tile([C, C], f32, tag="weT")
            nc.sync.dma_start(out=weT, in_=weT_d[:, :])
            ident = const.tile([C, C], f32, tag="ident")
            nc.sync.dma_start(out=ident, in_=ident_d[:, :])
            ident_r = const.tile([C, C], f32r, tag="ident_r")
            nc.scalar.copy(ident_r, ident)
            pwT = const.tile([C, C], f32, tag="pwT")
            nc.sync.dma_start(out=pwT, in_=pwT_d[:, :])
            dwd = []
            for s9 in range(9):
                t = const.tile([C, C], f32, tag=f"dwd{s9}", name=f"dwd{s9}")
                nc.sync.dma_start(out=t, in_=dwdiag_d[s9, :, :])
                dwd.append(t)
            dwb = const.tile([C, 1], f32, tag="dwb")
            nc.sync.dma_start(out=dwb, in_=dwb_d[:, :])
            dwa = const.tile([C, 1], f32, tag="dwa")
            nc.sync.dma_start(out=dwa, in_=dwa_d[:, :])
            pwb = const.tile([C, 1], f32, tag="pwb")
            nc.sync.dma_start(out=pwb, in_=pwb_d[:, :])
            pwa = const.tile([C, 1], f32, tag="pwa")
            nc.sync.dma_start(out=pwa, in_=pwa_d[:, :])
            # fp32r-rounded copies of matmul weights (walrus requires
            # fp32r operands to be produced as fp32r by a compute op)
            dwdr = []
            for s9 in range(9):
                t = const.tile([C, C], f32r, tag=f"dwdr{s9}",
                               name=f"dwdr{s9}")
                nc.scalar.copy(t, dwd[s9])
                dwdr.append(t)
            pwTr = const.tile([C, C], f32r, tag="pwTr")
            nc.scalar.copy(pwTr, pwT)

            prelu_ctr = [0]

            def prelu(out, in_, alpha):
                eng = nc.vector
                prelu_ctr[0] += 1
                eng.scalar_tensor_tensor(out=out, in0=in_, scalar=alpha,
                                         in1=in_, op0=AluOpType.mult,
                                         op1=AluOpType.max)

            for s in range(SPC):
                ex_sb = big.tile([C, H, W + 2], f32r, tag="ex",
                                 name=f"ex_s{s}")
                q_sb = big.tile([C, H, W + 2], f32r, tag="q",
                                name=f"q_s{s}")

                # ---- Phase A: load + transpose chunks + accumulate G^T ----
                with tc.tile_pool(name="psA", bufs=2, space="PSUM") as psA:
                    G_ps = psA.tile([C, C], f32, tag="G", bufs=1,
                                    name=f"G_s{s}")
                    for j in range(32):  # 512-col groups
                        if j % 4 == 0:
                            blk = slice(j * 512, j * 512 + 2048)
                            rows = slice(j * 4, j * 4 + 16)
                            nc.sync.dma_start(
                                out=ex_sb[:, rows, 1:W + 1],
                                in_=ex_d[s, :, blk].rearrange(
                                    "c (h w) -> c h w", w=W))
                            nc.sync.dma_start(
                                out=q_sb[:, rows, 1:W + 1],
                                in_=q_d[s, :, blk].rearrange(
                                    "c (h w) -> c h w", w=W))
                        exT_ps = psA.tile([C, 512], f32, tag="exT",
                                          name=f"exT_s{s}_{j}")
                        qT_ps = psA.tile([C, 512], f32, tag="qT",
                                         name=f"qT_s{s}_{j}")
                        for b in range(4):
                            row = 4 * j + b
                            bs = slice(b * C, (b + 1) * C)
                            nc.tensor.transpose(
                                exT_ps[:, bs],
                                ex_sb[:, row, 1:W + 1].bitcast(f32),
                                ident)
                            nc.tensor.transpose(
                                qT_ps[:, bs],
                                q_sb[:, row, 1:W + 1].bitcast(f32),
                                ident)
                        exT_sb = xtsb.tile([C, 512], f32, tag="exT_sb",
                                           name=f"exTsb_s{s}_{j}")
                        qT_sb = xtsb.tile([C, 512], f32, tag="qT_sb",
                                          name=f"qTsb_s{s}_{j}")
                        nc.scalar.copy(exT_sb, exT_ps)
                        nc.scalar.copy(qT_sb, qT_ps)
                        for b in range(4):
                            bs = slice(b * C, (b + 1) * C)
                            nc.tensor.matmul(G_ps, exT_sb[:, bs],
                                             qT_sb[:, bs],
                                             start=(j == 0 and b == 0),
                                             stop=(j == 31 and b == 3))
                    G_sb = smp.tile([C, C], f32, tag="G_sb", name=f"Gsb_s{s}")
                    nc.scalar.copy(G_sb, G_ps)

                # ---- small matmuls + softmaxes ----
                eTs = []
                rs = []
                with tc.tile_pool(name="psS", bufs=1, space="PSUM") as psS:
                    A_ps = psS.tile([C, C], f32, tag="A", name=f"A_s{s}")
                    nc.tensor.matmul(A_ps, G_sb, weT, start=True, stop=True)
                    AT_ps = psS.tile([C, C], f32, tag="AT", name=f"AT_s{s}")
                    nc.tensor.matmul(AT_ps, weT, G_sb, start=True, stop=True)
                    for bi, M_ps in enumerate((A_ps, AT_ps)):
                        mx = smp.tile([C, 1], f32, tag="mx",
                                      name=f"mx_s{s}_{bi}")
                        nc.vector.reduce_max(mx, M_ps, X)
                        mxn = smp.tile([C, 1], f32, tag="mxn",
                                       name=f"mxn_s{s}_{bi}")
                        nc.vector.tensor_scalar_mul(mxn, mx, -1.0)
                        e_sb = smp.tile([C, C], f32r, tag="e_sb",
                                        name=f"e_s{s}_{bi}")
                        sm = smp.tile([C, 1], f32, tag="sm",
                                      name=f"sm_s{s}_{bi}")
                        nc.scalar.activation(e_sb, M_ps, Exp, bias=mxn,
                                             scale=1.0, accum_out=sm)
                        r = smp.tile([C, 1], f32, tag=f"r{bi}",
                                     name=f"r_s{s}_{bi}")
                        nc.vector.reciprocal(r, sm)
                        eT_ps = psS.tile([C, C], f32r, tag="eT",
                                         name=f"eT_s{s}_{bi}")
                        nc.tensor.transpose(eT_ps, e_sb, ident_r)
                        eT_sb = smp.tile([C, C], f32r, tag=f"eT_sb{bi}",
                                         name=f"eTsb_s{s}_{bi}")
                        nc.scalar.copy(eT_sb, eT_ps)
                        eTs.append(eT_sb)
                        rs.append(r)

                # ---- branches: (query out), (exemplar out) ----
                branches = [(eTs[0], rs[0], ex_sb, q_sb, oq_d),
                            (eTs[1], rs[1], q_sb, ex_sb, oex_d)]
                for bi, (eT, rinv, rhs_big, res_big, out_d) in \
                        enumerate(branches):
                    with tc.tile_pool(name="psB", bufs=1, space="PSUM") \
                            as psB:
                        obuf = None
                        for t in range(NSTRIP):
                            base_row = STRIP * t - 1  # image row of j=0
                            jlo = 1 if t == 0 else 0
                            jhi = 9 if t == NSTRIP - 1 else 10
                            att_ps = psB.tile([C, 10, W], f32, tag="att",
                                              bufs=2,
                                              name=f"att_{s}_{bi}_{t}")
                            # attention matmuls, split at PSUM bank bounds
                            ja = jlo
                            for je in (4, 8, 10):
                                jb = min(jhi, je)
                                if jb <= ja:
                                    continue
                                nc.tensor.matmul(
                                    att_ps[:, ja:jb, :],
                                    eT,
                                    rhs_big[:, base_row + ja:base_row + jb,
                                            1:W + 1],
                                    start=True, stop=True)
                                ja = jb
                                if jb == jhi:
                                    break
                            y0 = y0p.tile([C, 10, W + 2], f32r, tag="y0",
                                          name=f"y0_{s}_{bi}_{t}")
                            nc.vector.memset(y0[:, :, 0:1].bitcast(f32),
                                             0.0)
                            nc.vector.memset(
                                y0[:, :, W + 1:W + 2].bitcast(f32), 0.0)
                            nc.vector.scalar_tensor_tensor(
                                out=y0[:, jlo:jhi, 1:W + 1],
                                in0=att_ps[:, jlo:jhi, :], scalar=rinv,
                                in1=res_big[:, base_row + jlo:base_row + jhi,
                                            1:W + 1].bitcast(f32),
                                op0=AluOpType.mult, op1=AluOpType.add)
                            if t % 2 == 0:
                                obuf = outp.tile([C, 2 * STRIP * W], f32,
                                                 tag="obuf",
                                                 name=f"ob_{s}_{bi}_{t}")
                            for wv in range(2):  # two 4-row psum windows
                                rbase = STRIP * t + 4 * wv
                                dw_ps = psB.tile([C, 4, W], f32, tag="dw",
                                                 bufs=1,
                                                 name=f"dw_{s}_{bi}_{t}{wv}")
                                si = 0
                                for dh in (-1, 0, 1):
                                    for dwx in (-1, 0, 1):
                                        rlo = max(rbase, -dh)
                                        rhi = min(rbase + 3, 127 - dh)
                                        a = rlo - rbase
                                        b2 = rhi - rbase + 1
                                        jin = rlo + dh - base_row
                                        nc.tensor.matmul(
                                            dw_ps[:, a:b2, :],
                                            dwdr[si],
                                            y0[:, jin:jin + (b2 - a),
                                               1 + dwx:1 + dwx + W],
                                            start=(si == 0), stop=(si == 8),
                                            skip_group_check=True)
                                        si += 1
                                y1 = convp.tile([C, 4, W], f32, tag="y1",
                                                name=f"y1_{s}_{bi}_{t}{wv}")
                                nc.scalar.activation(y1, dw_ps, Ident,
                                                     bias=dwb, scale=1.0)
                                y1p = convp.tile([C, 4, W + 4], f32r,
                                                 tag="y1p",
                                                 name=f"y1p_{s}_{bi}_{t}{wv}")
                                prelu(y1p[:, :, 0:W], y1, dwa)
                                pw_ps = psB.tile([C, 4 * W], f32, tag="pw",
                                                 bufs=1,
                                                 name=f"pw_{s}_{bi}_{t}{wv}")
                                nc.tensor.matmul(
                                    pw_ps, pwTr, y1p[:, :, 0:W],
                                    start=True, stop=True)
                                y2 = convp.tile([C, 4 * W], f32, tag="y2",
                                                name=f"y2_{s}_{bi}_{t}{wv}")
                                nc.scalar.activation(y2, pw_ps, Ident,
                                                     bias=pwb, scale=1.0)
                                off = (t % 2) * STRIP * W + wv * 4 * W
                                prelu(obuf[:, off:off + 4 * W], y2, pwa)
                            if t % 2 == 1:
                                nb = slice((t - 1) * STRIP * W,
                                           (t + 1) * STRIP * W)
                                nc.sync.dma_start(out=out_d[s, :, nb],
                                                  in_=obuf)
    nc.compile()
    return nc


def _get_nc():
    if 'nc' not in _cache:
        _cache['nc'] = _build()
    return _cache['nc']


def kernel(exemplar, query, W_e, dw_w, dw_gamma, dw_beta, dw_mean, dw_var,
           dw_alpha, pw_w, pw_gamma, pw_beta, pw_mean, pw_var, pw_alpha):
    from concourse.bass_utils import run_bass_kernel_spmd

    f = lambda a: np.asarray(a, dtype=np.float32)
    exemplar, query = f(exemplar), f(query)

    inv_dw = f(dw_gamma) / np.sqrt(f(dw_var) + np.float32(EPS))
    taps = f(dw_w)[:, 0, :, :] * inv_dw[:, None, None]   # [C,3,3]
    dwdiag = np.zeros((9, C, C), np.float32)
    for kh in range(3):
        for kw in range(3):
            np.fill_diagonal(dwdiag[kh * 3 + kw], taps[:, kh, kw])
    dwb = (f(dw_beta) - f(dw_mean) * inv_dw).reshape(C, 1)
    inv_pw = f(pw_gamma) / np.sqrt(f(pw_var) + np.float32(EPS))
    pwT = np.ascontiguousarray((f(pw_w)[:, :, 0, 0] * inv_pw[:, None]).T)
    pwb = (f(pw_beta) - f(pw_mean) * inv_pw).reshape(C, 1)

    shared = {
        "weT": np.ascontiguousarray(f(W_e).T),
        "ident": np.eye(C, dtype=np.float32),
        "dwdiag": dwdiag,
        "pwT": pwT,
        "dwb": dwb,
        "dwa": f(dw_alpha).reshape(C, 1),
        "pwb": pwb,
        "pwa": f(pw_alpha).reshape(C, 1),
    }
    exf = exemplar.reshape(B, C, N)
    qf = query.reshape(B, C, N)
    in_maps = []
    for c in range(NCORES):
        m = dict(shared)
        m["ex"] = np.ascontiguousarray(exf[SPC * c:SPC * (c + 1)])
        m["q"] = np.ascontiguousarray(qf[SPC * c:SPC * (c + 1)])
        in_maps.append(m)

    nc = _get_nc()
    res = run_bass_kernel_spmd(nc, in_maps, core_ids=list(range(NCORES)))

    oex = np.empty((B, C, N), np.float32)
    oq = np.empty((B, C, N), np.float32)
    for c in range(NCORES):
        oex[SPC * c:SPC * (c + 1)] = res.results[c]["oex"]
        oq[SPC * c:SPC * (c + 1)] = res.results[c]["oq"]
    return (oex.reshape(B, C, H, W), oq.reshape(B, C, H, W))


# revision 11
# speedup vs baseline: 1.0181x; 1.0181x over previous
"""Trainium2 Bass kernel for nn_CCorrM (co-correlation attention + dsconv).

Full-input contract: kernel(**inputs) takes the unsharded numpy inputs and
returns (exemplar_out, query_out), each [16, 128, 128, 128] float32.

Strategy: pure data parallel over batch B=16 across 8 NeuronCores
(2 samples per core); all params replicated.

Per-sample on-core pipeline (C=128, N=H*W=16384):
  G^T      = sum_n ex[:,n] q[:,n]^T           (PE transposes + fp32 matmuls)
  A        = G W_e^T, A^T = W_e G^T           (two small fp32 matmuls)
  softmax  rows of A and A^T (exp via ACT with accum row-sum; normalization
           folded into the attention-output eviction)
  y0       = softmax-att @ {ex|q} + residual  (fp32r matmuls, STT eviction)
  dwconv   9 diagonal fp32r matmuls accumulating in PSUM (3x3 depthwise,
           BN folded into tap weights host-side)
  prelu    = max(alpha*x, x) via scalar_tensor_tensor
  pwconv   1x1 conv = fp32r matmul (BN folded host-side), bias via ACT, prelu
"""

import sys

if '/opt/trn_rl_repo' not in sys.path:
    sys.path.insert(0, '/opt/trn_rl_repo')

import numpy as np

B, C, H, W = 16, 128, 128, 128
N = H * W
NCORES = 8
SPC = B // NCORES          # samples per core
STRIP = 8                  # output rows per strip
NSTRIP = H // STRIP
EPS = 1e-5

_cache = {}


def _build():
    import concourse.bacc as bacc
    import concourse.tile as tile
    from concourse import mybir
    from concourse.alu_op_type import AluOpType

    f32 = mybir.dt.float32
    f32r = mybir.dt.float32r
    Exp = mybir.ActivationFunctionType.Exp
    Ident = mybir.ActivationFunctionType.Identity
    X = mybir.AxisListType.X

    nc = bacc.Bacc("TRN2", target_bir_lowering=False, debug=False,
                   num_devices=NCORES)

    ex_d = nc.dram_tensor("ex", [SPC, C, N], f32r, kind="ExternalInput")
    q_d = nc.dram_tensor("q", [SPC, C, N], f32r, kind="ExternalInput")
    weT_d = nc.dram_tensor("weT", [C, C], f32, kind="ExternalInput")
    ident_d = nc.dram_tensor("ident", [C, C], f32, kind="ExternalInput")
    dwdiag_d = nc.dram_tensor("dwdiag", [9, C, C], f32, kind="ExternalInput")
    pwT_d = nc.dram_tensor("pwT", [C, C], f32, kind="ExternalInput")
    dwb_d = nc.dram_tensor("dwb", [C, 1], f32, kind="ExternalInput")
    dwa_d = nc.dram_tensor("dwa", [C, 1], f32, kind="ExternalInput")
    pwb_d = nc.dram_tensor("pwb", [C, 1], f32, kind="ExternalInput")
    pwa_d = nc.dram_tensor("pwa", [C, 1], f32, kind="ExternalInput")
    oex_d = nc.dram_tensor("oex", [SPC, C, N], f32, kind="ExternalOutput")
    oq_d = nc.dram_tensor("oq", [SPC, C, N], f32, kind="ExternalOutput")

    with tile.TileContext(nc) as tc:
        from contextlib import ExitStack
        with ExitStack() as ctx:
            const = ctx.enter_context(tc.tile_pool(name="const", bufs=1))
            big = ctx.enter_context(tc.tile_pool(name="big", bufs=1))
            xtsb = ctx.enter_context(tc.tile_pool(name="xtsb", bufs=3))
            smp = ctx.enter_context(tc.tile_pool(name="smp", bufs=2))
            y0p = ctx.enter_context(tc.tile_pool(name="y0p", bufs=2))
            convp = ctx.enter_context(tc.tile_pool(name="convp", bufs=2))
            outp = ctx.enter_context(tc.tile_pool(name="outp", bufs=2))

            weT = const.tile([C, C], f32, tag="weT")
            nc.sync.dma_start(out=weT, in_=weT_d[:, :])
            ident = const.tile([C, C], f32, tag="ident")
            nc.sync.dma_start(out=ident, in_=ident_d[:, :])
            ident_r = const.tile([C, C], f32r, tag="ident_r")
            nc.scalar.copy(ident_r, ident)
            pwT = const.tile([C, C], f32, tag="pwT")
            nc.sync.dma_start(out=pwT, in_=pwT_d[:, :])
            dwd = []
            for s9 in range(9):
                t = const.tile([C, C], f32, tag=f"dwd{s9}", name=f"dwd{s9}")
                nc.sync.dma_start(out=t, in_=dwdiag_d[s9, :, :])
                dwd.append(t)
            dwb = const.tile([C, 1], f32, tag="dwb")
            nc.sync.dma_start(out=dwb, in_=dwb_d[:, :])
            dwa = const.tile([C, 1], f32, tag="dwa")
            nc.sync.dma_start(out=dwa, in_=dwa_d[:, :])
            pwb = const.tile([C, 1], f32, tag="pwb")
            nc.sync.dma_start(out=pwb, in_=pwb_d[:, :])
            pwa = const.tile([C, 1], f32, tag="pwa")
            nc.sync.dma_start(out=pwa, in_=pwa_d[:, :])
            # fp32r-rounded copies of matmul weights (walrus requires
            # fp32r operands to be produced as fp32r by a compute op)
            dwdr = []
            for s9 in range(9):
                t = const.tile([C, C], f32r, tag=f"dwdr{s9}",
                               name=f"dwdr{s9}")
                nc.scalar.copy(t, dwd[s9])
                dwdr.append(t)
            pwTr = const.tile([C, C], f32r, tag="pwTr")
            nc.scalar.copy(pwTr, pwT)

            prelu_ctr = [0]

            def prelu(out, in_, alpha):
                eng = nc.vector
                prelu_ctr[0] += 1
                eng.scalar_tensor_tensor(out=out, in0=in_, scalar=alpha,
                                         in1=in_, op0=AluOpType.mult,
                                         op1=AluOpType.max)

            for s in range(SPC):
                ex_sb = big.tile([C, N], f32r, tag="ex", name=f"ex_s{s}")
                q_sb = big.tile([C, N], f32r, tag="q", name=f"q_s{s}")

                # ---- Phase A: load + transpose chunks + accumulate G^T ----
                with tc.tile_pool(name="psA", bufs=2, space="PSUM") as psA:
                    G_ps = psA.tile([C, C], f32, tag="G", bufs=1,
                                    name=f"G_s{s}")
                    for j in range(32):  # 512-col groups
                        if j % 4 == 0:
                            blk = slice(j * 512, j * 512 + 2048)
                            nc.sync.dma_start(out=ex_sb[:, blk],
                                              in_=ex_d[s, :, blk])
                            nc.sync.dma_start(out=q_sb[:, blk],
                                              in_=q_d[s, :, blk])
                        exT_ps = psA.tile([C, 512], f32, tag="exT",
                                          name=f"exT_s{s}_{j}")
                        qT_ps = psA.tile([C, 512], f32, tag="qT",
                                         name=f"qT_s{s}_{j}")
                        for b in range(4):
                            cs = slice((4 * j + b) * C, (4 * j + b + 1) * C)
                            bs = slice(b * C, (b + 1) * C)
                            nc.tensor.transpose(
                                exT_ps[:, bs], ex_sb[:, cs].bitcast(f32),
                                ident)
                            nc.tensor.transpose(
                                qT_ps[:, bs], q_sb[:, cs].bitcast(f32),
                                ident)
                        exT_sb = xtsb.tile([C, 512], f32, tag="exT_sb",
                                           name=f"exTsb_s{s}_{j}")
                        qT_sb = xtsb.tile([C, 512], f32, tag="qT_sb",
                                          name=f"qTsb_s{s}_{j}")
                        nc.scalar.copy(exT_sb, exT_ps)
                        nc.scalar.copy(qT_sb, qT_ps)
                        for b in range(4):
                            bs = slice(b * C, (b + 1) * C)
                            nc.tensor.matmul(G_ps, exT_sb[:, bs],
                                             qT_sb[:, bs],
                                             start=(j == 0 and b == 0),
                                             stop=(j == 31 and b == 3))
                    G_sb = smp.tile([C, C], f32, tag="G_sb", name=f"Gsb_s{s}")
                    nc.scalar.copy(G_sb, G_ps)

                # ---- small matmuls + softmaxes ----
                eTs = []
                rs = []
                with tc.tile_pool(name="psS", bufs=1, space="PSUM") as psS:
                    A_ps = psS.tile([C, C], f32, tag="A", name=f"A_s{s}")
                    nc.tensor.matmul(A_ps, G_sb, weT, start=True, stop=True)
                    AT_ps = psS.tile([C, C], f32, tag="AT", name=f"AT_s{s}")
                    nc.tensor.matmul(AT_ps, weT, G_sb, start=True, stop=True)
                    for bi, M_ps in enumerate((A_ps, AT_ps)):
                        mx = smp.tile([C, 1], f32, tag="mx",
                                      name=f"mx_s{s}_{bi}")
                        nc.vector.reduce_max(mx, M_ps, X)
                        mxn = smp.tile([C, 1], f32, tag="mxn",
                                       name=f"mxn_s{s}_{bi}")
                        nc.vector.tensor_scalar_mul(mxn, mx, -1.0)
                        e_sb = smp.tile([C, C], f32r, tag="e_sb",
                                        name=f"e_s{s}_{bi}")
                        sm = smp.tile([C, 1], f32, tag="sm",
                                      name=f"sm_s{s}_{bi}")
                        nc.scalar.activation(e_sb, M_ps, Exp, bias=mxn,
                                             scale=1.0, accum_out=sm)
                        r = smp.tile([C, 1], f32, tag=f"r{bi}",
                                     name=f"r_s{s}_{bi}")
                        nc.vector.reciprocal(r, sm)
                        eT_ps = psS.tile([C, C], f32r, tag="eT",
                                         name=f"eT_s{s}_{bi}")
                        nc.tensor.transpose(eT_ps, e_sb, ident_r)
                        eT_sb = smp.tile([C, C], f32r, tag=f"eT_sb{bi}",
                                         name=f"eTsb_s{s}_{bi}")
                        nc.scalar.copy(eT_sb, eT_ps)
                        eTs.append(eT_sb)
                        rs.append(r)

                # ---- branches: (query out), (exemplar out) ----
                branches = [(eTs[0], rs[0], ex_sb, q_sb, oq_d),
                            (eTs[1], rs[1], q_sb, ex_sb, oex_d)]
                for bi, (eT, rinv, rhs_big, res_big, out_d) in \
                        enumerate(branches):
                    with tc.tile_pool(name="psB", bufs=1, space="PSUM") \
                            as psB:
                        obuf = None
                        for t in range(NSTRIP):
                            base_row = STRIP * t - 1  # image row of j=0
                            jlo = 1 if t == 0 else 0
                            jhi = 9 if t == NSTRIP - 1 else 10
                            att_ps = psB.tile([C, 10, W], f32, tag="att",
                                              bufs=2,
                                              name=f"att_{s}_{bi}_{t}")
                            # attention matmuls, split at PSUM bank bounds
                            ja = jlo
                            for je in (4, 8, 10):
                                jb = min(jhi, je)
                                if jb <= ja:
                                    continue
                                nc.tensor.matmul(
                                    att_ps[:, ja:jb, :],
                                    eT,
                                    rhs_big[:, (base_row + ja) * W:
                                            (base_row + jb) * W],
                                    start=True, stop=True)
                                ja = jb
                                if jb == jhi:
                                    break
                            y0 = y0p.tile([C, 10, W + 2], f32r, tag="y0",
                                          name=f"y0_{s}_{bi}_{t}")
                            nc.vector.memset(y0[:, :, 0:1].bitcast(f32),
                                             0.0)
                            nc.vector.memset(
                                y0[:, :, W + 1:W + 2].bitcast(f32), 0.0)
                            nc.vector.scalar_tensor_tensor(
                                out=y0[:, jlo:jhi, 1:W + 1],
                                in0=att_ps[:, jlo:jhi, :], scalar=rinv,
                                in1=res_big[:, (base_row + jlo) * W:
                                            (base_row + jhi) * W].rearrange(
                                    "c (j w) -> c j w", w=W).bitcast(f32),
                                op0=AluOpType.mult, op1=AluOpType.add)
                            if t % 2 == 0:
                                obuf = outp.tile([C, 2 * STRIP * W], f32,
                                                 tag="obuf",
                                                 name=f"ob_{s}_{bi}_{t}")
                            for wv in range(2):  # two 4-row psum windows
                                rbase = STRIP * t + 4 * wv
                                dw_ps = psB.tile([C, 4, W], f32, tag="dw",
                                                 bufs=1,
                                                 name=f"dw_{s}_{bi}_{t}{wv}")
                                si = 0
                                for dh in (-1, 0, 1):
                                    for dwx in (-1, 0, 1):
                                        rlo = max(rbase, -dh)
                                        rhi = min(rbase + 3, 127 - dh)
                                        a = rlo - rbase
                                        b2 = rhi - rbase + 1
                                        jin = rlo + dh - base_row
                                        nc.tensor.matmul(
                                            dw_ps[:, a:b2, :],
                                            dwdr[si],
                                            y0[:, jin:jin + (b2 - a),
                                               1 + dwx:1 + dwx + W],
                                            start=(si == 0), stop=(si == 8),
                                            skip_group_check=True)
                                        si += 1
                                y1 = convp.tile([C, 4, W], f32, tag="y1",
                                                name=f"y1_{s}_{bi}_{t}{wv}")
                                nc.scalar.activation(y1, dw_ps, Ident,
                                                     bias=dwb, scale=1.0)
                                y1p = convp.tile([C, 4, W + 4], f32r,
                                                 tag="y1p",
                                                 name=f"y1p_{s}_{bi}_{t}{wv}")
                                prelu(y1p[:, :, 0:W], y1, dwa)
                                pw_ps = psB.tile([C, 4 * W], f32, tag="pw",
                                                 bufs=1,
                                                 name=f"pw_{s}_{bi}_{t}{wv}")
                                nc.tensor.matmul(
                                    pw_ps, pwTr, y1p[:, :, 0:W],
                                    start=True, stop=True)
                                y2 = convp.tile([C, 4 * W], f32, tag="y2",
                                                name=f"y2_{s}_{bi}_{t}{wv}")
                                nc.scalar.activation(y2, pw_ps, Ident,
                                                     bias=pwb, scale=1.0)
                                off = (t % 2) * STRIP * W + wv * 4 * W
                                prelu(obuf[:, off:off + 4 * W], y2, pwa)
                            if t % 2 == 1:
                                nb = slice((t - 1) * STRIP * W,
                                           (t + 1) * STRIP * W)
                                nc.sync.dma_start(out=out_d[s, :, nb],
                                                  in_=obuf)
    nc.compile()
    return nc


def _get_nc():
    if 'nc' not in _cache:
        _cache['nc'] = _build()
    return _cache['nc']


def kernel(exemplar, query, W_e, dw_w, dw_gamma, dw_beta, dw_mean, dw_var,
           dw_alpha, pw_w, pw_gamma, pw_beta, pw_mean, pw_var, pw_alpha):
    from concourse.bass_utils import run_bass_kernel_spmd

    f = lambda a: np.asarray(a, dtype=np.float32)
    exemplar, query = f(exemplar), f(query)

    inv_dw = f(dw_gamma) / np.sqrt(f(dw_var) + np.float32(EPS))
    taps = f(dw_w)[:, 0, :, :] * inv_dw[:, None, None]   # [C,3,3]
    dwdiag = np.zeros((9, C, C), np.float32)
    for kh in range(3):
        for kw in range(3):
            np.fill_diagonal(dwdiag[kh * 3 + kw], taps[:, kh, kw])
    dwb = (f(dw_beta) - f(dw_mean) * inv_dw).reshape(C, 1)
    inv_pw = f(pw_gamma) / np.sqrt(f(pw_var) + np.float32(EPS))
    pwT = np.ascontiguousarray((f(pw_w)[:, :, 0, 0] * inv_pw[:, None]).T)
    pwb = (f(pw_beta) - f(pw_mean) * inv_pw).reshape(C, 1)

    shared = {
        "weT": np.ascontiguousarray(f(W_e).T),
        "ident": np.eye(C, dtype=np.float32),
        "dwdiag": dwdiag,
        "pwT": pwT,
        "dwb": dwb,
        "dwa": f(dw_alpha).reshape(C, 1),
        "pwb": pwb,
        "pwa": f(pw_alpha).reshape(C, 1),
    }
    exf = exemplar.reshape(B, C, N)
    qf = query.reshape(B, C, N)
    in_maps = []
    for c in range(NCORES):
        m = dict(shared)
        m["ex"] = np.ascontiguousarray(exf[SPC * c:SPC * (c + 1)])
        m["q"] = np.ascontiguousarray(qf[SPC * c:SPC * (c + 1)])
        in_maps.append(m)

    nc = _get_nc()
    res = run_bass_kernel_spmd(nc, in_maps, core_ids=list(range(NCORES)))

    oex = np.empty((B, C, N), np.float32)
    oq = np.empty((B, C, N), np.float32)
    for c in range(NCORES):
        oex[SPC * c:SPC * (c + 1)] = res.results[c]["oex"]
        oq[SPC * c:SPC * (c + 1)] = res.results[c]["oq"]
    return (oex.reshape(B, C, H, W), oq.reshape(B, C, H, W))


# revision 12
# speedup vs baseline: 1.0594x; 1.0406x over previous
"""Trainium2 Bass kernel for nn_CCorrM (co-correlation attention + dsconv).

Full-input contract: kernel(**inputs) takes the unsharded numpy inputs and
returns (exemplar_out, query_out), each [16, 128, 128, 128] float32.

Strategy: pure data parallel over batch B=16 across 8 NeuronCores
(2 samples per core); all params replicated.

Per-sample on-core pipeline (C=128, N=H*W=16384):
  G^T      = sum_n ex[:,n] q[:,n]^T           (PE transposes + fp32 matmuls)
  A        = G W_e^T, A^T = W_e G^T           (two small fp32 matmuls)
  softmax  rows of A and A^T (exp via ACT with accum row-sum; normalization
           folded into the attention-output eviction)
  y0       = softmax-att @ {ex|q} + residual  (fp32r matmuls, STT eviction)
  dwconv   9 diagonal fp32r matmuls accumulating in PSUM (3x3 depthwise,
           BN folded into tap weights host-side)
  prelu    = max(alpha*x, x) via scalar_tensor_tensor
  pwconv   1x1 conv = fp32r matmul (BN folded host-side), bias via ACT, prelu
"""

import sys

if '/opt/trn_rl_repo' not in sys.path:
    sys.path.insert(0, '/opt/trn_rl_repo')

import numpy as np

B, C, H, W = 16, 128, 128, 128
N = H * W
NCORES = 8
SPC = B // NCORES          # samples per core
STRIP = 8                  # output rows per strip
NSTRIP = H // STRIP
EPS = 1e-5

_cache = {}


def _build():
    import concourse.bacc as bacc
    import concourse.tile as tile
    from concourse import mybir
    from concourse.alu_op_type import AluOpType

    f32 = mybir.dt.float32
    f32r = mybir.dt.float32r
    Exp = mybir.ActivationFunctionType.Exp
    Ident = mybir.ActivationFunctionType.Identity
    X = mybir.AxisListType.X

    nc = bacc.Bacc("TRN2", target_bir_lowering=False, debug=False,
                   num_devices=NCORES)

    ex_d = nc.dram_tensor("ex", [SPC, C, N], f32r, kind="ExternalInput")
    q_d = nc.dram_tensor("q", [SPC, C, N], f32r, kind="ExternalInput")
    weT_d = nc.dram_tensor("weT", [C, C], f32, kind="ExternalInput")
    ident_d = nc.dram_tensor("ident", [C, C], f32, kind="ExternalInput")
    dwdiag_d = nc.dram_tensor("dwdiag", [9, C, C], f32, kind="ExternalInput")
    pwT_d = nc.dram_tensor("pwT", [C, C], f32, kind="ExternalInput")
    dwb_d = nc.dram_tensor("dwb", [C, 1], f32, kind="ExternalInput")
    dwa_d = nc.dram_tensor("dwa", [C, 1], f32, kind="ExternalInput")
    pwb_d = nc.dram_tensor("pwb", [C, 1], f32, kind="ExternalInput")
    pwa_d = nc.dram_tensor("pwa", [C, 1], f32, kind="ExternalInput")
    oex_d = nc.dram_tensor("oex", [SPC, C, N], f32, kind="ExternalOutput")
    oq_d = nc.dram_tensor("oq", [SPC, C, N], f32, kind="ExternalOutput")

    with tile.TileContext(nc) as tc:
        from contextlib import ExitStack
        with ExitStack() as ctx:
            const = ctx.enter_context(tc.tile_pool(name="const", bufs=1))
            big = ctx.enter_context(tc.tile_pool(name="big", bufs=1))
            xtsb = ctx.enter_context(tc.tile_pool(name="xtsb", bufs=3))
            smp = ctx.enter_context(tc.tile_pool(name="smp", bufs=2))
            y0p = ctx.enter_context(tc.tile_pool(name="y0p", bufs=2))
            convp = ctx.enter_context(tc.tile_pool(name="convp", bufs=2))
            outp = ctx.enter_context(tc.tile_pool(name="outp", bufs=2))

            weT = const.tile([C, C], f32, tag="weT")
            nc.sync.dma_start(out=weT, in_=weT_d[:, :])
            ident = const.tile([C, C], f32, tag="ident")
            nc.sync.dma_start(out=ident, in_=ident_d[:, :])
            ident_r = const.tile([C, C], f32r, tag="ident_r")
            nc.scalar.copy(ident_r, ident)
            pwT = const.tile([C, C], f32, tag="pwT")
            nc.sync.dma_start(out=pwT, in_=pwT_d[:, :])
            dwd = []
            for s9 in range(9):
                t = const.tile([C, C], f32, tag=f"dwd{s9}", name=f"dwd{s9}")
                nc.sync.dma_start(out=t, in_=dwdiag_d[s9, :, :])
                dwd.append(t)
            dwb = const.tile([C, 1], f32, tag="dwb")
            nc.sync.dma_start(out=dwb, in_=dwb_d[:, :])
            dwa = const.tile([C, 1], f32, tag="dwa")
            nc.sync.dma_start(out=dwa, in_=dwa_d[:, :])
            pwb = const.tile([C, 1], f32, tag="pwb")
            nc.sync.dma_start(out=pwb, in_=pwb_d[:, :])
            pwa = const.tile([C, 1], f32, tag="pwa")
            nc.sync.dma_start(out=pwa, in_=pwa_d[:, :])
            # fp32r-rounded copies of matmul weights (walrus requires
            # fp32r operands to be produced as fp32r by a compute op)
            dwdr = []
            for s9 in range(9):
                t = const.tile([C, C], f32r, tag=f"dwdr{s9}",
                               name=f"dwdr{s9}")
                nc.scalar.copy(t, dwd[s9])
                dwdr.append(t)
            pwTr = const.tile([C, C], f32r, tag="pwTr")
            nc.scalar.copy(pwTr, pwT)

            prelu_ctr = [0]

            def prelu(out, in_, alpha):
                eng = nc.vector
                prelu_ctr[0] += 1
                eng.scalar_tensor_tensor(out=out, in0=in_, scalar=alpha,
                                         in1=in_, op0=AluOpType.mult,
                                         op1=AluOpType.max)

            for s in range(SPC):
                ex_sb = big.tile([C, N], f32r, tag="ex", name=f"ex_s{s}")
                q_sb = big.tile([C, N], f32r, tag="q", name=f"q_s{s}")

                # ---- Phase A: load + transpose chunks + accumulate G^T ----
                with tc.tile_pool(name="psA", bufs=2, space="PSUM") as psA:
                    G_ps = psA.tile([C, C], f32, tag="G", bufs=1,
                                    name=f"G_s{s}")
                    for j in range(32):  # 512-col groups
                        if j % 4 == 0:
                            blk = slice(j * 512, j * 512 + 2048)
                            nc.sync.dma_start(out=ex_sb[:, blk],
                                              in_=ex_d[s, :, blk])
                            nc.sync.dma_start(out=q_sb[:, blk],
                                              in_=q_d[s, :, blk])
                        exT_ps = psA.tile([C, 512], f32, tag="exT",
                                          name=f"exT_s{s}_{j}")
                        qT_ps = psA.tile([C, 512], f32, tag="qT",
                                         name=f"qT_s{s}_{j}")
                        for b in range(4):
                            cs = slice((4 * j + b) * C, (4 * j + b + 1) * C)
                            bs = slice(b * C, (b + 1) * C)
                            nc.tensor.transpose(
                                exT_ps[:, bs], ex_sb[:, cs].bitcast(f32),
                                ident)
                            nc.tensor.transpose(
                                qT_ps[:, bs], q_sb[:, cs].bitcast(f32),
                                ident)
                        exT_sb = xtsb.tile([C, 512], f32, tag="exT_sb",
                                           name=f"exTsb_s{s}_{j}")
                        qT_sb = xtsb.tile([C, 512], f32, tag="qT_sb",
                                          name=f"qTsb_s{s}_{j}")
                        nc.scalar.copy(exT_sb, exT_ps)
                        nc.scalar.copy(qT_sb, qT_ps)
                        for b in range(4):
                            bs = slice(b * C, (b + 1) * C)
                            nc.tensor.matmul(G_ps, exT_sb[:, bs],
                                             qT_sb[:, bs],
                                             start=(j == 0 and b == 0),
                                             stop=(j == 31 and b == 3))
                    G_sb = smp.tile([C, C], f32, tag="G_sb", name=f"Gsb_s{s}")
                    nc.scalar.copy(G_sb, G_ps)

                # ---- small matmuls + softmaxes ----
                eTs = []
                rs = []
                with tc.tile_pool(name="psS", bufs=1, space="PSUM") as psS:
                    A_ps = psS.tile([C, C], f32, tag="A", name=f"A_s{s}")
                    nc.tensor.matmul(A_ps, G_sb, weT, start=True, stop=True)
                    AT_ps = psS.tile([C, C], f32, tag="AT", name=f"AT_s{s}")
                    nc.tensor.matmul(AT_ps, weT, G_sb, start=True, stop=True)
                    for bi, M_ps in enumerate((A_ps, AT_ps)):
                        mx = smp.tile([C, 1], f32, tag="mx",
                                      name=f"mx_s{s}_{bi}")
                        nc.vector.reduce_max(mx, M_ps, X)
                        mxn = smp.tile([C, 1], f32, tag="mxn",
                                       name=f"mxn_s{s}_{bi}")
                        nc.vector.tensor_scalar_mul(mxn, mx, -1.0)
                        e_sb = smp.tile([C, C], f32r, tag="e_sb",
                                        name=f"e_s{s}_{bi}")
                        sm = smp.tile([C, 1], f32, tag="sm",
                                      name=f"sm_s{s}_{bi}")
                        nc.scalar.activation(e_sb, M_ps, Exp, bias=mxn,
                                             scale=1.0, accum_out=sm)
                        r = smp.tile([C, 1], f32, tag=f"r{bi}",
                                     name=f"r_s{s}_{bi}")
                        nc.vector.reciprocal(r, sm)
                        eT_ps = psS.tile([C, C], f32r, tag="eT",
                                         name=f"eT_s{s}_{bi}")
                        nc.tensor.transpose(eT_ps, e_sb, ident_r)
                        eT_sb = smp.tile([C, C], f32r, tag=f"eT_sb{bi}",
                                         name=f"eTsb_s{s}_{bi}")
                        nc.scalar.copy(eT_sb, eT_ps)
                        eTs.append(eT_sb)
                        rs.append(r)

                # ---- branches: (query out), (exemplar out) ----
                branches = [(eTs[0], rs[0], ex_sb, q_sb, oq_d),
                            (eTs[1], rs[1], q_sb, ex_sb, oex_d)]
                for bi, (eT, rinv, rhs_big, res_big, out_d) in \
                        enumerate(branches):
                    with tc.tile_pool(name="psB", bufs=1, space="PSUM") \
                            as psB:
                        def strip_geom(t):
                            base_row = STRIP * t - 1
                            jlo = 1 if t == 0 else 0
                            jhi = 9 if t == NSTRIP - 1 else 10
                            return base_row, jlo, jhi

                        def emit_att(t):
                            base_row, jlo, jhi = strip_geom(t)
                            att_ps = psB.tile([C, 10, W], f32, tag="att",
                                              bufs=2, uniquify=True,
                                              name=f"att_{s}_{bi}_{t}")
                            ja = jlo
                            for je in (4, 8, 10):
                                jb = min(jhi, je)
                                if jb <= ja:
                                    continue
                                nc.tensor.matmul(
                                    att_ps[:, ja:jb, :],
                                    eT,
                                    rhs_big[:, (base_row + ja) * W:
                                            (base_row + jb) * W],
                                    start=True, stop=True)
                                ja = jb
                                if jb == jhi:
                                    break
                            return att_ps

                        def emit_stt(t, att_ps):
                            base_row, jlo, jhi = strip_geom(t)
                            y0 = y0p.tile([C, 10, W + 2], f32r, tag="y0",
                                          uniquify=True,
                                          name=f"y0_{s}_{bi}_{t}")
                            nc.vector.memset(y0[:, :, 0:1].bitcast(f32),
                                             0.0)
                            nc.vector.memset(
                                y0[:, :, W + 1:W + 2].bitcast(f32), 0.0)
                            for (ha, hb) in ((jlo, 6), (6, jhi)):
                                nc.vector.scalar_tensor_tensor(
                                    out=y0[:, ha:hb, 1:W + 1],
                                    in0=att_ps[:, ha:hb, :], scalar=rinv,
                                    in1=res_big[:, (base_row + ha) * W:
                                                (base_row + hb) * W]
                                    .rearrange("c (j w) -> c j w", w=W)
                                    .bitcast(f32),
                                    op0=AluOpType.mult, op1=AluOpType.add)
                            return y0

                        def emit_dw(t, wv, y0):
                            base_row, jlo, jhi = strip_geom(t)
                            rbase = STRIP * t + 4 * wv
                            dw_ps = psB.tile([C, 4, W], f32, tag="dw",
                                             bufs=1, uniquify=True,
                                             name=f"dw_{s}_{bi}_{t}{wv}")
                            si = 0
                            for dh in (-1, 0, 1):
                                for dwx in (-1, 0, 1):
                                    rlo = max(rbase, -dh)
                                    rhi = min(rbase + 3, 127 - dh)
                                    a = rlo - rbase
                                    b2 = rhi - rbase + 1
                                    jin = rlo + dh - base_row
                                    nc.tensor.matmul(
                                        dw_ps[:, a:b2, :],
                                        dwdr[si],
                                        y0[:, jin:jin + (b2 - a),
                                           1 + dwx:1 + dwx + W],
                                        start=(si == 0), stop=(si == 8),
                                        skip_group_check=True)
                                    si += 1
                            y1 = convp.tile([C, 4, W], f32, tag="y1",
                                            uniquify=True,
                                            name=f"y1_{s}_{bi}_{t}{wv}")
                            nc.scalar.activation(y1, dw_ps, Ident,
                                                 bias=dwb, scale=1.0)
                            y1p = convp.tile([C, 4, W + 4], f32r,
                                             tag="y1p", bufs=3,
                                             uniquify=True,
                                             name=f"y1p_{s}_{bi}_{t}{wv}")
                            prelu(y1p[:, :, 0:W], y1, dwa)
                            return y1p

                        obufs = {}

                        def emit_pw(t, wv, y1p):
                            if t % 2 == 0 and wv == 0:
                                obufs[t // 2] = outp.tile(
                                    [C, 2 * STRIP * W], f32, tag="obuf",
                                    uniquify=True,
                                    name=f"ob_{s}_{bi}_{t}")
                            obuf = obufs[t // 2]
                            pw_ps = psB.tile([C, 4 * W], f32, tag="pw",
                                             bufs=1, uniquify=True,
                                             name=f"pw_{s}_{bi}_{t}{wv}")
                            nc.tensor.matmul(
                                pw_ps, pwTr, y1p[:, :, 0:W],
                                start=True, stop=True)
                            y2 = convp.tile([C, 4 * W], f32, tag="y2",
                                            uniquify=True,
                                            name=f"y2_{s}_{bi}_{t}{wv}")
                            nc.scalar.activation(y2, pw_ps, Ident,
                                                 bias=pwb, scale=1.0)
                            off = (t % 2) * STRIP * W + wv * 4 * W
                            prelu(obuf[:, off:off + 4 * W], y2, pwa)
                            if t % 2 == 1 and wv == 1:
                                nb = slice((t - 1) * STRIP * W,
                                           (t + 1) * STRIP * W)
                                nc.sync.dma_start(out=out_d[s, :, nb],
                                                  in_=obuf)

                        from collections import deque
                        pending = deque()
                        att_t = {0: emit_att(0)}
                        for t in range(NSTRIP):
                            y0 = emit_stt(t, att_t.pop(t))
                            if t + 1 < NSTRIP:
                                att_t[t + 1] = emit_att(t + 1)
                            for wv in range(2):
                                y1p = emit_dw(t, wv, y0)
                                if pending:
                                    emit_pw(*pending.popleft())
                                pending.append((t, wv, y1p))
                        while pending:
                            emit_pw(*pending.popleft())
    nc.compile()
    return nc


def _get_nc():
    if 'nc' not in _cache:
        _cache['nc'] = _build()
    return _cache['nc']


def kernel(exemplar, query, W_e, dw_w, dw_gamma, dw_beta, dw_mean, dw_var,
           dw_alpha, pw_w, pw_gamma, pw_beta, pw_mean, pw_var, pw_alpha):
    from concourse.bass_utils import run_bass_kernel_spmd

    f = lambda a: np.asarray(a, dtype=np.float32)
    exemplar, query = f(exemplar), f(query)

    inv_dw = f(dw_gamma) / np.sqrt(f(dw_var) + np.float32(EPS))
    taps = f(dw_w)[:, 0, :, :] * inv_dw[:, None, None]   # [C,3,3]
    dwdiag = np.zeros((9, C, C), np.float32)
    for kh in range(3):
        for kw in range(3):
            np.fill_diagonal(dwdiag[kh * 3 + kw], taps[:, kh, kw])
    dwb = (f(dw_beta) - f(dw_mean) * inv_dw).reshape(C, 1)
    inv_pw = f(pw_gamma) / np.sqrt(f(pw_var) + np.float32(EPS))
    pwT = np.ascontiguousarray((f(pw_w)[:, :, 0, 0] * inv_pw[:, None]).T)
    pwb = (f(pw_beta) - f(pw_mean) * inv_pw).reshape(C, 1)

    shared = {
        "weT": np.ascontiguousarray(f(W_e).T),
        "ident": np.eye(C, dtype=np.float32),
        "dwdiag": dwdiag,
        "pwT": pwT,
        "dwb": dwb,
        "dwa": f(dw_alpha).reshape(C, 1),
        "pwb": pwb,
        "pwa": f(pw_alpha).reshape(C, 1),
    }
    exf = exemplar.reshape(B, C, N)
    qf = query.reshape(B, C, N)
    in_maps = []
    for c in range(NCORES):
        m = dict(shared)
        m["ex"] = np.ascontiguousarray(exf[SPC * c:SPC * (c + 1)])
        m["q"] = np.ascontiguousarray(qf[SPC * c:SPC * (c + 1)])
        in_maps.append(m)

    nc = _get_nc()
    res = run_bass_kernel_spmd(nc, in_maps, core_ids=list(range(NCORES)))

    oex = np.empty((B, C, N), np.float32)
    oq = np.empty((B, C, N), np.float32)
    for c in range(NCORES):
        oex[SPC * c:SPC * (c + 1)] = res.results[c]["oex"]
        oq[SPC * c:SPC * (c + 1)] = res.results[c]["oq"]
    return (oex.reshape(B, C, H, W), oq.reshape(B, C, H, W))
